# revision 1
# baseline (speedup 1.0000x reference)
"""AqlmOFTLinear distributed Trainium2 kernel (8 NeuronCores).

Strategy:
  - Data-parallel over tokens for x (2048 tokens/core, host pre-transposed to
    feature-major), tensor-parallel dequant of the AQLM weight (512 out rows
    per core) with a chunked AllGather of the rotated weight V = BD @ W^T.
  - AQLM dequant on-device via gpsimd.dma_gather from a paired codebook table
    [32768, 128]bf16 where entry q = [cb[q] | cb[q+32768] | pad]; the int16
    index is (code & 32767). Gathers round-robin over all 4 SWDGE queues.
  - The half-select + row scale are folded into ONE dense tensor_tensor
    multiply at PSUM evacuation time: both gathered halves are PE-transposed
    to [i'', o] layout and multiplied by host-built ABt coefficients
    (lo: s*(1-m), hi: s*m); the lo+hi sum happens for free in the PSUM
    accumulation of the rotation matmul V = Q^T_blockdiag @ (Wlo^T + Whi^T).
  - OFT Cayley on-device: Q^T = (I+S^8)(I+S^4)(I+S^2)(I-S)^2 with the 128
    32x32 blocks packed 4-up into block-diagonal 128x128 f32 matmuls.
  - Main matmul in bf16: out^T[o, t] = sum_i V[i, o] * x[t, i], PSUM-accum
    over 32 k-chunks, bias fused into the PSUM->SBUF evacuation, f32 output
    written via SWDGE cast-DMA.
"""

import os
import sys

import numpy as np

sys.path.insert(0, "/opt/trn_rl_repo")

import ml_dtypes

BF16 = ml_dtypes.bfloat16

N_CORES = 8
IN_F = 4096
OUT_F = 4096
TOK = 16384
TOK_PC = TOK // N_CORES          # 2048 tokens per core
OUT_PC = OUT_F // N_CORES        # 512 out-features per core
GROUP = 8
N_G = IN_F // GROUP              # 512 groups
HALF_CB = 32768                  # paired table entries
ELEM = 128                       # bf16 elems per table entry (256B)
N_IC = IN_F // 128               # 32 input-feature chunks
GC_G = 32                        # groups per gather call -> two 128-i'' chunks
CALLS_PER_OC = N_G // GC_G       # 16 calls per o-chunk
NIDX = 128 * GC_G                # 4096 indices per gather call
NQ = 4                           # SWDGE queues

_BUILD_CACHE = {}
LAST_RESULT = None


def _patched_dma_gather():
    """dma_gather with the elem_size %256 assert relaxed: the 256B constraint
    is xbar-transpose-only; natural-mode 32B elements work on HW (verified)
    and cut gather traffic 8x by skipping the table pad."""
    import inspect
    import re

    import concourse.bass as cb

    fsrc = inspect.getsource(type(cb.Bass().gpsimd).dma_gather)
    fsrc = fsrc.replace(
        "elem_size_bytes > 0 and elem_size_bytes % 256 == 0", "elem_size_bytes > 0"
    )
    fsrc = re.sub(r"^    def dma_gather", "def dma_gather", fsrc, flags=re.M)
    fsrc = re.sub(r"\n    ", "\n", fsrc)
    ns = dict(vars(cb))
    exec(compile(fsrc, "patched_dma_gather", "exec"), ns)
    return ns["dma_gather"]


def _build_nc():
    from concourse import bacc, mybir, tile

    dma_gather32 = _patched_dma_gather()

    f32 = mybir.dt.float32
    bf16 = mybir.dt.bfloat16
    i16 = mybir.dt.int16

    nc = bacc.Bacc(num_devices=N_CORES, num_swdge_queues=NQ)

    # ---- DRAM parameters (per-core shards supplied via in_maps) ----
    xT_d = nc.declare_dram_parameter("xT", [IN_F, TOK_PC], f32, isOutput=False)
    table_d = nc.declare_dram_parameter("table", [HALF_CB, ELEM], bf16, isOutput=False)
    idx_d = nc.declare_dram_parameter(
        "idx", [4 * CALLS_PER_OC, 128, NIDX // 16], i16, isOutput=False
    )
    # ABo[call, o_local, plane, g_local*8+j] select+scale multipliers
    abt_d = nc.declare_dram_parameter("abt", [64, 128, 512], bf16, isOutput=False)
    bias_d = nc.declare_dram_parameter("bias_p", [128, 32], f32, isOutput=False)
    rbd_d = nc.declare_dram_parameter("rbd", [N_IC, 128, 128], f32, isOutput=False)
    identf_d = nc.declare_dram_parameter("identf", [128, 128], f32, isOutput=False)
    identb_d = nc.declare_dram_parameter("identb", [128, 128], bf16, isOutput=False)
    outT_d = nc.declare_dram_parameter("outT", [OUT_F, TOK_PC], bf16, isOutput=True)

    # ---- internal DRAM for the collective (4 o-chunks of 128) ----
    cc_in = [nc.dram_tensor(f"cc_in{j}", [IN_F, 128], bf16) for j in range(4)]
    cc_out = [
        nc.dram_tensor(f"cc_out{j}", [N_CORES * IN_F, 128], bf16, addr_space="Shared")
        for j in range(4)
    ]
    rg = [list(range(N_CORES))]

    with tile.TileContext(nc) as tc:
        with (
            tc.tile_pool(name="const", bufs=1) as constp,
            tc.tile_pool(name="qt", bufs=1) as qtp,
            tc.tile_pool(name="xh", bufs=1) as xhp,
            tc.tile_pool(name="vs", bufs=2) as vsp,
            tc.tile_pool(name="ob", bufs=2) as obp,
            tc.tile_pool(name="cay", bufs=5) as cayp,
            tc.tile_pool(name="deq", bufs=1) as deqp,
            tc.tile_pool(name="deq2", bufs=2) as deq2p,
            tc.tile_pool(name="deqg", bufs=4) as deqgp,
            tc.tile_pool(name="psA", bufs=2, space="PSUM") as psA,
            tc.tile_pool(name="psV", bufs=2, space="PSUM") as psVp,
            tc.tile_pool(name="psB", bufs=2, space="PSUM") as psB,
        ):
            # ---- constants ----
            identf = constp.tile([128, 128], f32)
            nc.sync.dma_start(out=identf[:], in_=identf_d[:])
            identb = constp.tile([128, 128], bf16)
            nc.sync.dma_start(out=identb[:], in_=identb_d[:])
            bias_sb = constp.tile([128, 32], f32)
            nc.sync.dma_start(out=bias_sb[:], in_=bias_d[:])
            ident4 = constp.tile([128, 4, 128], f32)
            for k in range(4):
                nc.vector.tensor_copy(ident4[:, k, :], identf[:])

            qt_sb = qtp.tile([128, N_IC, 128], bf16)  # Q^T block-diag chunks
            nidx_reg = nc.gpsimd.to_reg(NIDX)  # shared across all gather calls

            # ================= Phase A0: Cayley =================
            for g in range(8):
                rbd_sb = cayp.tile([128, 4, 128], f32, tag="cay")
                nc.sync.dma_start(
                    out=rbd_sb[:],
                    in_=rbd_d[g * 4:(g + 1) * 4, :, :].rearrange("c p f -> p c f"),
                )
                psT = psA.tile([128, 4, 128], f32, tag="ps")
                for k in range(4):
                    nc.tensor.transpose(psT[:, k, :], rbd_sb[:, k, :], identf[:])
                tmp = cayp.tile([128, 4, 128], f32, tag="cay")
                nc.vector.tensor_scalar_mul(tmp[:], rbd_sb[:], 0.5)
                S = cayp.tile([128, 4, 128], f32, tag="cay")
                nc.vector.scalar_tensor_tensor(
                    S[:], psT[:], -0.5, tmp[:],
                    mybir.AluOpType.mult, mybir.AluOpType.add,
                )
                negS = cayp.tile([128, 4, 128], f32, tag="cay")
                nc.vector.tensor_scalar_mul(negS[:], S[:], -1.0)
                P1T = cayp.tile([128, 4, 128], f32, tag="cay")  # I - S
                nc.vector.scalar_tensor_tensor(
                    P1T[:], S[:], -1.0, ident4[:],
                    mybir.AluOpType.mult, mybir.AluOpType.add,
                )
                P1 = cayp.tile([128, 4, 128], f32, tag="cay")  # I + S
                nc.vector.tensor_tensor(P1[:], S[:], ident4[:], mybir.AluOpType.add)
                ps2 = psA.tile([128, 4, 128], f32, tag="ps")
                for k in range(4):
                    nc.tensor.matmul(ps2[:, k, :], negS[:, k, :], S[:, k, :])
                S2 = cayp.tile([128, 4, 128], f32, tag="cay")
                nc.vector.tensor_copy(S2[:], ps2[:])
                P2 = cayp.tile([128, 4, 128], f32, tag="cay")  # I + S^2
                nc.vector.tensor_tensor(P2[:], S2[:], ident4[:], mybir.AluOpType.add)
                ps4 = psA.tile([128, 4, 128], f32, tag="ps")
                for k in range(4):
                    nc.tensor.matmul(ps4[:, k, :], S2[:, k, :], S2[:, k, :])
                S4 = cayp.tile([128, 4, 128], f32, tag="cay")
                nc.vector.tensor_copy(S4[:], ps4[:])
                P3 = cayp.tile([128, 4, 128], f32, tag="cay")  # I + S^4
                nc.vector.tensor_tensor(P3[:], S4[:], ident4[:], mybir.AluOpType.add)
                ps8 = psA.tile([128, 4, 128], f32, tag="ps")
                for k in range(4):
                    nc.tensor.matmul(ps8[:, k, :], S4[:, k, :], S4[:, k, :])
                P4 = cayp.tile([128, 4, 128], f32, tag="cay")  # I + S^8
                nc.vector.scalar_tensor_tensor(
                    P4[:], ps8[:], 1.0, ident4[:],
                    mybir.AluOpType.mult, mybir.AluOpType.add,
                )
                # T1 = (I-S)^2 — Q^T needs the extra (I-S) numerator factor
                psT1 = psA.tile([128, 4, 128], f32, tag="ps")
                for k in range(4):
                    nc.tensor.matmul(psT1[:, k, :], P1[:, k, :], P1T[:, k, :])
                T1 = cayp.tile([128, 4, 128], f32, tag="cay")
                nc.vector.tensor_copy(T1[:], psT1[:])
                psb1 = psA.tile([128, 4, 128], f32, tag="ps")
                for k in range(4):
                    nc.tensor.matmul(psb1[:, k, :], P2[:, k, :], T1[:, k, :])
                B1 = cayp.tile([128, 4, 128], f32, tag="cay")
                nc.vector.tensor_copy(B1[:], psb1[:])
                psb2 = psA.tile([128, 4, 128], f32, tag="ps")
                for k in range(4):
                    nc.tensor.matmul(psb2[:, k, :], P3[:, k, :], B1[:, k, :])
                B2 = cayp.tile([128, 4, 128], f32, tag="cay")
                nc.vector.tensor_copy(B2[:], psb2[:])
                psb3 = psA.tile([128, 4, 128], f32, tag="ps")
                for k in range(4):
                    nc.tensor.matmul(psb3[:, k, :], P4[:, k, :], B2[:, k, :])
                nc.vector.tensor_copy(qt_sb[:, g * 4:(g + 1) * 4, :], psb3[:])

            # ---- x load early: SWDGE cast f32 -> bf16 ----
            xh = xhp.tile([128, N_IC, TOK_PC], bf16)
            for ic in range(N_IC):
                nc.gpsimd.dma_start(
                    out=xh[:, ic, :], in_=xT_d[ic * 128:(ic + 1) * 128, :]
                )

            # ====== Phase A1 + B interleaved ======
            # Main-matmul chains of s4=oc-1 are woven between gather calls of
            # o-chunk oc so the PE's static program order alternates
            # (gather-paced dequant transposes) with (ready main matmuls).
            vsb_hold = {}

            def emit_main_chain(s4, chain):
                r_, tb = divmod(chain, 2)
                if tb == 0:
                    t = vsp.tile([128, N_IC, 128], bf16, tag="vsb")
                    nc.sync.dma_start(
                        out=t[:],
                        in_=cc_out[s4][r_ * IN_F:(r_ + 1) * IN_F, :].rearrange(
                            "(ic p) o -> p ic o", p=128
                        ),
                    )
                    vsb_hold[(s4, r_)] = t
                t = vsb_hold[(s4, r_)]
                ps = psB.tile([128, 1024], f32, tag="psb")
                for ic in range(N_IC):
                    nc.tensor.matmul(
                        ps[:, 0:512], t[:, ic, :],
                        xh[:, ic, tb * 1024:tb * 1024 + 512],
                        start=(ic == 0), stop=(ic == N_IC - 1),
                    )
                    nc.tensor.matmul(
                        ps[:, 512:1024], t[:, ic, :],
                        xh[:, ic, tb * 1024 + 512:(tb + 1) * 1024],
                        start=(ic == 0), stop=(ic == N_IC - 1),
                    )
                ob = obp.tile([128, 1024], bf16, tag="ob")
                s_glob = r_ * 4 + s4
                nc.vector.tensor_scalar_add(
                    ob[:], ps[:], bias_sb[:, s_glob:s_glob + 1]
                )
                nc.sync.dma_start(
                    out=outT_d[
                        s_glob * 128:(s_glob + 1) * 128,
                        tb * 1024:(tb + 1) * 1024,
                    ],
                    in_=ob[:],
                )

            qn = 0
            ags_emitted = [0]
            chain_no = [0]
            for oc in range(4):
                # wt2[:, 2*ic+plane, :] = scaled W^T plane tiles [i'', o]
                wt2 = deqp.tile([128, 2 * N_IC, 128], bf16, tag="wt2")
                for cg in range(CALLS_PER_OC):  # one call covers 2 ic chunks
                    call = oc * CALLS_PER_OC + cg
                    idx_sb = deq2p.tile([128, NIDX // 16], i16, tag="idx")
                    nc.sync.dma_start(out=idx_sb[:], in_=idx_d[call, :, :])
                    G = deqgp.tile([128, GC_G, 2 * GROUP], bf16, tag="G")
                    dma_gather32(
                        nc.gpsimd, G[:], table_d[:, 0:2 * GROUP], idx_sb[:],
                        num_idxs=NIDX, num_idxs_reg=nidx_reg,
                        elem_size=2 * GROUP, elem_step=ELEM,
                        single_packet=False, queue_num=qn % NQ,
                    )
                    qn += 1
                    abo = deq2p.tile([128, 2, GC_G, GROUP], bf16, tag="abo")
                    nc.sync.dma_start(
                        out=abo[:],
                        in_=abt_d[call, :, :].rearrange(
                            "p (pl g j) -> p pl g j", pl=2, j=GROUP
                        ),
                    )
                    # fused select+scale: Gs[pl] = G-plane * coeff (compact out)
                    Gs = deqgp.tile([128, 2, 2 * ELEM], bf16, tag="Gs")
                    nc.vector.tensor_tensor(
                        Gs[:].rearrange("p pl (g j) -> p pl g j", j=GROUP),
                        G[:].rearrange("p g (pl j) -> p pl g j", j=GROUP),
                        abo[:],
                        mybir.AluOpType.mult,
                    )
                    psT2 = psA.tile([128, 4, 128], bf16, tag="ps")
                    for sub in range(2):
                        for pl in range(2):
                            nc.tensor.transpose(
                                psT2[:, 2 * sub + pl, :],
                                Gs[:, pl, sub * 128:(sub + 1) * 128],
                                identb[:],
                            )
                    nc.vector.tensor_copy(wt2[:, cg * 4:(cg + 1) * 4, :], psT2[:])
                    # defer AG(oc-1) to cg==3 of this oc: by then the previous
                    # rotation has drained, so the Pool-issued AG trigger does
                    # not bubble the gather stream
                    if oc >= 1 and cg == 3:
                        po = oc - 1
                        nc.gpsimd.collective_compute(
                            "AllGather",
                            mybir.AluOpType.bypass,
                            replica_groups=rg,
                            ins=[cc_in[po][:, :].opt()],
                            outs=[cc_out[po][:, :].opt()],
                        )
                        ags_emitted[0] += 1
                    if chain_no[0] < 16 * ags_emitted[0]:
                        s4c, cc_ = divmod(chain_no[0], 16)
                        emit_main_chain(s4c, cc_)
                        chain_no[0] += 1
                # rotate: V[i, o] = sum_i'' BD[i,i''] (Wlo^T + Whi^T)[i'', o]
                for icg in range(8):
                    psV = psVp.tile([128, 4, 128], f32, tag="psv")
                    for q in range(4):
                        ic = icg * 4 + q
                        nc.tensor.matmul(
                            psV[:, q, :], qt_sb[:, ic, :], wt2[:, 2 * ic, :],
                            start=True, stop=False,
                        )
                        nc.tensor.matmul(
                            psV[:, q, :], qt_sb[:, ic, :], wt2[:, 2 * ic + 1, :],
                            start=False, stop=True,
                        )
                    vout = deq2p.tile([128, 4, 128], bf16, tag="vout")
                    nc.vector.tensor_copy(vout[:], psV[:])
                    for q in range(4):
                        ic = icg * 4 + q
                        nc.sync.dma_start(
                            out=cc_in[oc][ic * 128:(ic + 1) * 128, :],
                            in_=vout[:, q, :],
                        )

            # tail: AG of the last o-chunk + all remaining main chains
            nc.gpsimd.collective_compute(
                "AllGather",
                mybir.AluOpType.bypass,
                replica_groups=rg,
                ins=[cc_in[3][:, :].opt()],
                outs=[cc_out[3][:, :].opt()],
            )
            while chain_no[0] < 64:
                s4c, cc_ = divmod(chain_no[0], 16)
                emit_main_chain(s4c, cc_)
                chain_no[0] += 1
    nc.compile()
    return nc


def _host_prep(x, oft_r, codes, codebooks, scales, bias):
    """Shard + repack all inputs for the 8 cores."""
    xt = np.ascontiguousarray(np.asarray(x, dtype=np.float32).reshape(TOK, IN_F))
    codes2 = np.asarray(codes, dtype=np.int64)[:, :, 0]        # [4096, 512]
    cb = np.asarray(codebooks, dtype=np.float32)[0]            # [65536, 8]
    scales = np.asarray(scales, dtype=np.float32).reshape(OUT_F)
    bias = np.asarray(bias, dtype=np.float32).reshape(OUT_F)
    R = np.asarray(oft_r, dtype=np.float32)                    # [128, 32, 32]

    table = np.zeros((HALF_CB, ELEM), dtype=BF16)
    table[:, 0:GROUP] = cb[:HALF_CB].astype(BF16)
    table[:, GROUP:2 * GROUP] = cb[HALF_CB:].astype(BF16)

    rbd = np.zeros((N_IC, 128, 128), dtype=np.float32)
    Rb = R.reshape(N_IC, 4, 32, 32)
    for a in range(4):
        rbd[:, a * 32:(a + 1) * 32, a * 32:(a + 1) * 32] = Rb[:, a]

    identf = np.eye(128, dtype=np.float32)
    identb = np.eye(128, dtype=BF16)
    bias_p = np.ascontiguousarray(bias.reshape(32, 128).T)     # [128, 32]

    in_maps = []
    for r in range(N_CORES):
        xT = np.ascontiguousarray(xt[r * TOK_PC:(r + 1) * TOK_PC].T)  # [4096, 2048]
        c = codes2[r * OUT_PC:(r + 1) * OUT_PC]                # [512 o, 512 g]
        idx14 = (c & 32767).astype(np.int16)
        m = (c >> 15).astype(np.float32)                       # 0/1 mask
        # gather call (oc, gc): idx stream n = gl*128 + ol
        idx_c = idx14.reshape(4, 128, CALLS_PER_OC, GC_G)      # [oc, ol, gc, gl]
        stream = np.ascontiguousarray(idx_c.transpose(0, 2, 3, 1)).reshape(
            4 * CALLS_PER_OC, NIDX
        )
        wrapped = stream.reshape(4 * CALLS_PER_OC, NIDX // 16, 16).transpose(0, 2, 1)
        idx_dram = np.ascontiguousarray(
            np.broadcast_to(
                wrapped[:, None, :, :], (4 * CALLS_PER_OC, 8, 16, NIDX // 16)
            ).reshape(4 * CALLS_PER_OC, 128, NIDX // 16)
        )
        # ABo multipliers: lo plane s*(1-m), hi plane s*m, in [o, pl, g] layout
        sc = scales[r * OUT_PC:(r + 1) * OUT_PC]               # [512]
        A = sc[:, None] * (1.0 - m)                            # [512 o, 512 g]
        B = sc[:, None] * m
        AB = np.stack([A, B], axis=0)                          # [pl, o, g]
        ABg = AB.reshape(2, 4, 128, CALLS_PER_OC, GC_G)        # [pl, oc, ol, gc, gl]
        tmpv = ABg.transpose(1, 3, 2, 0, 4)                    # [oc, gc, ol, pl, gl]
        abt = np.repeat(tmpv[..., None], GROUP, axis=5)        # [oc, gc, ol, pl, gl, j]
        abt = np.ascontiguousarray(abt).reshape(
            4 * CALLS_PER_OC, 128, 2 * GC_G * GROUP
        ).astype(BF16)
        in_maps.append(
            dict(
                xT=xT,
                table=table,
                idx=idx_dram,
                abt=abt,
                bias_p=bias_p,
                rbd=rbd,
                identf=identf,
                identb=identb,
            )
        )
    return in_maps


def kernel(x, oft_r, codes, codebooks, scales, bias):
    global LAST_RESULT
    from concourse.bass_utils import run_bass_kernel_spmd

    if "nc" not in _BUILD_CACHE:
        _BUILD_CACHE["nc"] = _build_nc()
    nc = _BUILD_CACHE["nc"]

    in_maps = _host_prep(x, oft_r, codes, codebooks, scales, bias)
    trace = bool(int(os.environ.get("AQLM_TRACE", "0")))
    res = run_bass_kernel_spmd(nc, in_maps, core_ids=list(range(N_CORES)), trace=trace)
    LAST_RESULT = res

    out = np.empty((TOK, OUT_F), dtype=np.float32)
    for r in range(N_CORES):
        out[r * TOK_PC:(r + 1) * TOK_PC, :] = res.results[r]["outT"].T.astype(np.float32)
    return out.reshape(4, 4096, 4096).astype(np.asarray(x).dtype)

